# revision 1
# baseline (speedup 1.0000x reference)
"""Causal attention block (B=4, S=2048, D=1024, H=16) on 8 Trainium2 NeuronCores.

Sharding: core c = (batch b = c//2, head-group hg = c%2 of 8 heads).
Each core computes QKV projection for its batch restricted to its heads'
columns, causal flash-style attention for its 8 heads, and a partial output
projection (its heads' rows of W_proj). Host sums the two partial outputs
per batch pair and returns the full [4, 2048, 1024] result.

Layout choices (per core):
  - x arrives pre-transposed as xT [1024, 2048] so the embedding dim (the
    matmul contraction dim) is the SBUF partition dim.
  - q, k are produced transposed: qT/kT [512 cols, 2048 tokens] stored as
    [128, 4, 2048] tiles; head h lives in tile chunk h//2, partitions
    (h%2)*64..+64. 1/sqrt(hd) folded into W_q on the host.
  - v is produced in natural [token, col] orientation as [128, 16, 8, 65]
    (key-block, head, 64 v-cols + a ones column for softmax denominators).
  - scores are computed transposed, sT[k, q] = kT_block.T @ qT, exp'd with no
    max subtraction (scores are ~N(0,1); fp32 exp cannot overflow), causal
    diagonal masked with affine_select, then att@v accumulates over key
    blocks in PSUM; the ones column yields the denominator row.
  - normalization: reciprocal of the denominator row, partition_broadcast,
    multiply, staged to the out-projection lhsT layout via SBUF->SBUF DMA.
"""

import numpy as np

import concourse.bass as bass
import concourse.mybir as mybir
import concourse.tile as tile
from concourse import bacc
from concourse.bass_utils import run_bass_kernel_spmd
from concourse.masks import make_upper_triangular

F32 = mybir.dt.float32
F32R = mybir.dt.float32r
EMB = 1024
HEADS = 16
HD = 64
B = 4
S = 2048
NCORES = 8
HPC = 8           # heads per core
CD = HPC * HD     # 512 cols per core for each of q/k/v
NKB = S // 128    # 16 key blocks
NQC = S // 512    # 4 query chunks

_EXP = mybir.ActivationFunctionType.Exp


def _build_module(debug_dump=False):
    nc = bacc.Bacc("TRN2", target_bir_lowering=False, debug=False)
    xT = nc.declare_dram_parameter("xT", [EMB, S], F32R, isOutput=False)
    wq = nc.declare_dram_parameter("wq", [EMB, CD], F32R, isOutput=False)
    wk = nc.declare_dram_parameter("wk", [EMB, CD], F32R, isOutput=False)
    wv = nc.declare_dram_parameter("wv", [EMB, CD], F32R, isOutput=False)
    wp = nc.declare_dram_parameter("wp", [CD, EMB], F32R, isOutput=False)
    bias = nc.declare_dram_parameter("bias", [1, EMB], F32, isOutput=False)
    ones = nc.declare_dram_parameter("ones", [NKB, HPC], F32R, isOutput=False)
    y = nc.declare_dram_parameter("y", [S, EMB], F32, isOutput=True)
    dbg = None
    if debug_dump:
        dbg = {
            "qt": nc.declare_dram_parameter("dbg_qt", [128, 4, S], F32, isOutput=True),
            "kt": nc.declare_dram_parameter("dbg_kt", [128, 4, S], F32, isOutput=True),
            "vx": nc.declare_dram_parameter(
                "dbg_vx", [128, NKB, HPC, HD + 1], F32, isOutput=True
            ),
            "oT": nc.declare_dram_parameter(
                "dbg_oT", [NQC, 128, 4, 512], F32, isOutput=True
            ),
            "den": nc.declare_dram_parameter(
                "dbg_den", [NQC, HPC, 1, 512], F32, isOutput=True
            ),
            "bc": nc.declare_dram_parameter(
                "dbg_bc", [NQC, HPC, 64, 512], F32, isOutput=True
            ),
            "un": nc.declare_dram_parameter(
                "dbg_un", [NQC, HPC, HD, 512], F32, isOutput=True
            ),
        }

    with tile.TileContext(nc) as tc:
        _body(tc, nc, xT, wq, wk, wv, wp, bias, ones, y, dbg)
    nc.compile()
    return nc


def _body(tc, nc, xT, wq, wk, wv, wp, bias, ones, y, dbg=None):
    from contextlib import ExitStack

    with ExitStack() as ctx:
        persist = ctx.enter_context(tc.tile_pool(name="persist", bufs=1))
        qt = persist.tile([128, 4, S], F32R, tag="qt")
        kt = persist.tile([128, 4, S], F32R, tag="kt")
        vx = persist.tile([128, NKB, HPC, HD + 1], F32R, tag="vx")

        # ones column for denominators (DMA-broadcast from host input; memset
        # cannot produce float32r)
        nc.sync.dma_start(
            out=vx[:, :, :, HD : HD + 1], in_=ones[:].partition_broadcast(128)
        )
        # causal mask for diagonal blocks: tri[p, f] = 1.0 iff f >= p
        tri = persist.tile([128, 128], F32, tag="tri")
        make_upper_triangular(nc, tri[:], val=1.0, diag=True)
        tri2 = persist.tile([128, 256], F32, tag="tri2")
        nc.gpsimd.memset(tri2[:, 0:128], 0.0)
        make_upper_triangular(nc, tri2[:, 128:256], val=1.0, diag=True)

        # ---------------- Phase 1: QKV projections ----------------
        with ExitStack() as p1:
            xt_pool = p1.enter_context(tc.tile_pool(name="xt", bufs=2))
            w_pool = p1.enter_context(tc.tile_pool(name="w", bufs=4))
            wv_pool = p1.enter_context(tc.tile_pool(name="wvp", bufs=1))
            qkv_ps = p1.enter_context(
                tc.tile_pool(name="qkvps", bufs=4, space="PSUM")
            )

            wv_sb = wv_pool.tile([128, 8, CD], F32R, tag="wv")

            for half in range(2):
                t0 = half * 1024
                xt_sb = xt_pool.tile([128, 8, 1024], F32R, tag="xt")
                # load the n=0 token halves of every chunk first: the first
                # matmul group needs only those, halving the startup DMA gate
                for n2 in range(2):
                    for kc in range(8):
                        c0 = t0 + n2 * 512
                        nc.sync.dma_start(
                            out=xt_sb[:, kc, n2 * 512 : (n2 + 1) * 512],
                            in_=xT[kc * 128 : (kc + 1) * 128, c0 : c0 + 512],
                        )
                # qT and kT (transposed outputs)
                for qk, wdram, dst in ((0, wq, qt), (1, wk, kt)):
                    for m in range(4):
                        wt = w_pool.tile([128, 8, 128], F32R, tag="w")
                        nc.scalar.dma_start(
                            out=wt[:],
                            in_=wdram[:, m * 128 : (m + 1) * 128].rearrange(
                                "(c p) m -> p c m", p=128
                            ),
                        )
                        for n in range(2):
                            ps = qkv_ps.tile([128, 512], F32, tag="qkvps")
                            for kc in range(8):
                                nc.tensor.matmul(
                                    ps[:],
                                    lhsT=(wt[:, kc, :]),
                                    rhs=(xt_sb[:, kc, n * 512 : (n + 1) * 512]),
                                    start=(kc == 0),
                                    stop=(kc == 7),
                                )
                            col = t0 + n * 512
                            nc.vector.tensor_copy(
                                out=dst[:, m, col : col + 512], in_=ps[:]
                            )
                # v (natural orientation, strided into vx)
                if half == 0:
                    for kc in range(8):
                        nc.gpsimd.dma_start(
                            out=wv_sb[:, kc, :],
                            in_=wv[kc * 128 : (kc + 1) * 128, :],
                        )
                for tc8 in range(8):
                    tg = half * 8 + tc8
                    ps = qkv_ps.tile([128, 512], F32, tag="qkvps")
                    for kc in range(8):
                        nc.tensor.matmul(
                            ps[:],
                            lhsT=(xt_sb[:, kc, tc8 * 128 : (tc8 + 1) * 128]),
                            rhs=(wv_sb[:, kc, :]),
                            start=(kc == 0),
                            stop=(kc == 7),
                        )
                    nc.vector.tensor_copy(
                        out=vx[:, tg, :, 0:HD],
                        in_=ps[:].rearrange("p (h d) -> p h d", h=HPC),
                    )

        if dbg is not None:
            nc.sync.dma_start(out=dbg["qt"][:], in_=qt[:])
            nc.sync.dma_start(out=dbg["kt"][:], in_=kt[:])
            nc.sync.dma_start(out=dbg["vx"][:], in_=vx[:])

        # ---------------- Phase 2+3: attention + output projection ----------------
        with ExitStack() as p2:
            misc = p2.enter_context(tc.tile_pool(name="misc", bufs=1))
            s_pool = p2.enter_context(tc.tile_pool(name="sps", bufs=3, space="PSUM"))
            outT_pool = p2.enter_context(
                tc.tile_pool(name="outTps", bufs=3, space="PSUM")
            )
            y_pool = p2.enter_context(tc.tile_pool(name="yps", bufs=2, space="PSUM"))
            e_pool = p2.enter_context(tc.tile_pool(name="es", bufs=5))
            r_pool = p2.enter_context(tc.tile_pool(name="recip", bufs=3))
            b_pool = p2.enter_context(tc.tile_pool(name="bcast", bufs=3))
            st_pool = p2.enter_context(tc.tile_pool(name="stage", bufs=4))
            oT_pool = p2.enter_context(tc.tile_pool(name="oT", bufs=2))
            ysb_pool = p2.enter_context(tc.tile_pool(name="ysb", bufs=3))

            scr_pool = p2.enter_context(
                tc.tile_pool(name="scr", bufs=6, space="DRAM")
            )
            wp_sb = misc.tile([128, 4, EMB], F32R, tag="wp")
            nc.sync.dma_start(
                out=wp_sb[:], in_=wp[:].rearrange("(c p) e -> p c e", p=128)
            )
            bias_sb = misc.tile([128, 1, EMB], F32, tag="bias")
            nc.sync.dma_start(out=bias_sb[:], in_=bias[:].partition_broadcast(128))
            # ones row at partition 64 (same base partition as the denominator
            # row) for the PE-broadcast used on the final head
            onescol = misc.tile([65, 64], F32, tag="onescol")
            nc.sync.dma_start(
                out=onescol[64:65, 0:64], in_=ones[0:8, 0:8].bitcast(F32)
            )

            def make_y_group(oT_prev, qc_prev, tc4, ncol):
                def emit():
                    row = qc_prev * 512 + tc4 * 128
                    y_ps = y_pool.tile([128, 512], F32, tag="y")
                    for kc in range(4):
                        nc.tensor.matmul(
                            y_ps[:],
                            lhsT=(oT_prev[:, kc, tc4 * 128 : (tc4 + 1) * 128]),
                            rhs=(wp_sb[:, kc, ncol * 512 : (ncol + 1) * 512]),
                            start=(kc == 0),
                            stop=(kc == 3),
                        )
                    y_sb = ysb_pool.tile([128, 512], F32, tag="ysb")
                    nc.vector.tensor_add(
                        y_sb[:],
                        y_ps[:],
                        bias_sb[:, 0, ncol * 512 : (ncol + 1) * 512],
                    )
                    nc.sync.dma_start(
                        out=y[row : row + 128, ncol * 512 : (ncol + 1) * 512],
                        in_=y_sb[:],
                    )

                return emit

            deferred_y = []
            for qc in range(NQC):
                oT = oT_pool.tile([128, 4, 512], F32R, tag="oT")
                kb_max = 4 * qc + 4
                head_order = (
                    (1, 3, 5, 7, 0, 2, 4, 6) if qc == NQC - 1 else tuple(range(HPC))
                )
                for h in head_order:
                    m, po = h // 2, (h % 2) * 64
                    outT_ps = outT_pool.tile([HD + 1, 512], F32, tag="outT")
                    # previous chunk's output projection fills the PE while
                    # this head's exp chain runs on the scalar engine
                    if deferred_y:
                        deferred_y.pop(0)()

                    def emit_av(pending_infos, pending_es):
                        for j, kb, q0, nq, diag in pending_infos:
                            nc.tensor.matmul(
                                out=outT_ps[:, q0:512],
                                lhsT=(vx[:, kb, h, :]),
                                rhs=(pending_es[:, 0:nq]),
                                start=(kb == 0),
                                stop=(kb == kb_max - 1),
                            )

                    pending = None
                    for kb in range(kb_max):
                        r = kb * 128 - qc * 512
                        q0 = max(r, 0)
                        nq = 512 - q0
                        pad = r >= 0 and nq < 256
                        if pad:
                            # keep the moving dim >= 256 (fp32r runs 4x
                            # slower below that); mask the extra columns
                            q0, nq = 256, 256
                        s_ps = s_pool.tile([128, 512], F32, tag="s")
                        es = e_pool.tile([128, 512], F32R, tag="es")
                        nc.tensor.matmul(
                            out=s_ps[:, 0:nq],
                            lhsT=(kt[po : po + 64, m, kb * 128 : (kb + 1) * 128]),
                            rhs=(qt[po : po + 64, m, qc * 512 + q0 : (qc + 1) * 512]),
                            start=True,
                            stop=True,
                        )
                        nc.scalar.activation(
                            out=es[:, 0:nq], in_=s_ps[:, 0:nq], func=_EXP
                        )
                        if pad:
                            nc.vector.tensor_mul(es[:, 0:256], es[:, 0:256], tri2[:])
                        elif r >= 0:
                            nc.vector.tensor_mul(es[:, 0:128], es[:, 0:128], tri[:])
                        # av matmuls run one block behind so the PE never
                        # waits on the exp of the block it just produced
                        if pending is not None:
                            emit_av(*pending)
                        pending = ([(0, kb, q0, nq, r >= 0)], es)
                    if pending is not None:
                        emit_av(*pending)
                    # normalize: divide by denominator row (row HD)
                    recip = r_pool.tile([HD + 1, 512], F32, tag="recip")
                    nc.vector.reciprocal(
                        recip[HD : HD + 1, :], outT_ps[HD : HD + 1, :]
                    )
                    bcast = b_pool.tile([64, 512], F32, tag="bcast")
                    if qc == NQC - 1 and h == head_order[-1]:
                        # final head: its normalize chain is fully exposed at
                        # the kernel tail, so broadcast via an idle-PE matmul
                        # (ones column x reciprocal row) instead of the
                        # higher-latency DRAM-bounce DMA pair
                        bc_ps = s_pool.tile([64, 512], F32, tag="s")
                        nc.tensor.matmul(
                            out=bc_ps[:],
                            lhsT=onescol[64:65, :],
                            rhs=recip[HD : HD + 1, :],
                            start=True,
                            stop=True,
                        )
                        nc.vector.tensor_copy(out=bcast[:], in_=bc_ps[:])
                    else:
                        scr = scr_pool.tile([1, 512], F32, tag="scr")
                        nc.sync.dma_start(out=scr[:], in_=recip[HD : HD + 1, :])
                        nc.sync.dma_start(
                            out=bcast[:], in_=scr[0:1, :].partition_broadcast(64)
                        )
                    if po == 0:
                        nc.vector.tensor_mul(
                            oT[0:HD, m, :], outT_ps[0:HD, :], bcast[:]
                        )
                    else:
                        stage = st_pool.tile([64, 512], F32R, tag="stage")
                        nc.vector.tensor_mul(stage[:], outT_ps[0:HD, :], bcast[:])
                        nc.sync.dma_start(out=oT[po : po + 64, m, :], in_=stage[:])
                    if dbg is not None:
                        den_sb = st_pool.tile([HD + 1, 512], F32, tag="dbgden")
                        nc.vector.tensor_copy(
                            out=den_sb[HD : HD + 1, :], in_=outT_ps[HD : HD + 1, :]
                        )
                        nc.sync.dma_start(
                            out=dbg["den"][qc, h], in_=den_sb[HD : HD + 1, :]
                        )
                        un_sb = st_pool.tile([HD, 512], F32, tag="dbgun")
                        nc.vector.tensor_copy(out=un_sb[:], in_=outT_ps[0:HD, :])
                        nc.sync.dma_start(out=dbg["un"][qc, h], in_=un_sb[:])
                        nc.sync.dma_start(out=dbg["bc"][qc, h], in_=bcast[:])

                if dbg is not None:
                    nc.sync.dma_start(out=dbg["oT"][qc], in_=oT[:])

                # defer this chunk's output projection into the next chunk's
                # head loop (emitted one group per head)
                assert not deferred_y
                deferred_y = [
                    make_y_group(oT, qc, tc4, ncol)
                    for tc4 in range(4)
                    for ncol in range(2)
                ]
            for emit in deferred_y:
                emit()


_MODULE = None


def _get_module():
    global _MODULE
    if _MODULE is None:
        _MODULE = _build_module()
    return _MODULE


def _make_in_maps(x, W_qkv, W_proj, b_proj):
    scale = np.float32(1.0 / np.sqrt(HD))
    bias_half = (np.asarray(b_proj, dtype=np.float32) * 0.5).reshape(1, EMB)
    in_maps = []
    for c in range(NCORES):
        b, hg = c // 2, c % 2
        cols = slice(hg * CD, (hg + 1) * CD)
        in_maps.append(
            {
                "xT": np.ascontiguousarray(np.asarray(x[b], dtype=np.float32).T),
                "wq": np.ascontiguousarray(W_qkv[:, 0:EMB][:, cols]) * scale,
                "wk": np.ascontiguousarray(W_qkv[:, EMB : 2 * EMB][:, cols]),
                "wv": np.ascontiguousarray(W_qkv[:, 2 * EMB : 3 * EMB][:, cols]),
                "wp": np.ascontiguousarray(W_proj[cols, :]),
                "bias": bias_half,
                "ones": np.ones((NKB, HPC), dtype=np.float32),
            }
        )
    return in_maps


def kernel(x, W_qkv, W_proj, b_proj, _trace=False, _trace_kwargs=None):
    x = np.asarray(x, dtype=np.float32)
    W_qkv = np.asarray(W_qkv, dtype=np.float32)
    W_proj = np.asarray(W_proj, dtype=np.float32)
    b_proj = np.asarray(b_proj, dtype=np.float32)

    nc = _get_module()
    in_maps = _make_in_maps(x, W_qkv, W_proj, b_proj)
    res = run_bass_kernel_spmd(
        nc, in_maps, list(range(NCORES)), trace=_trace, **(_trace_kwargs or {})
    )
    out = np.empty((B, S, EMB), dtype=np.float32)
    for b in range(B):
        out[b] = res.results[2 * b]["y"] + res.results[2 * b + 1]["y"]
    if _trace:
        return out, res
    return out



# revision 10
# speedup vs baseline: 1.0224x; 1.0224x over previous
"""Causal attention block (B=4, S=2048, D=1024, H=16) on 8 Trainium2 NeuronCores.

Sharding: core c = (batch b = c//2, head-group hg = c%2 of 8 heads).
Each core computes QKV projection for its batch restricted to its heads'
columns, causal flash-style attention for its 8 heads, and a partial output
projection (its heads' rows of W_proj). Host sums the two partial outputs
per batch pair and returns the full [4, 2048, 1024] result.

All matmul operands are bf16 (PSUM accumulation stays fp32): on TRN2 the PE
processes one moving row per cycle regardless of dtype, but bf16 halves DMA
bytes and lifts the fp32r moving<256 penalty, and 2-byte DVE ops run 2x.

Layout choices (per core):
  - x arrives pre-transposed as xT [1024, 2048] so the embedding dim (the
    matmul contraction dim) is the SBUF partition dim.
  - q, k are produced transposed: qT/kT [512 cols, 2048 tokens] stored as
    [128, 4, 2048] tiles; head h lives in tile chunk h//2, partitions
    (h%2)*64..+64. 1/sqrt(hd) folded into W_q on the host.
  - v is produced in natural [token, col] orientation as [128, 16, 8, 65]
    (key-block, head, 64 v-cols + a ones column for softmax denominators).
  - scores are computed transposed, sT[k, q] = kT_block.T @ qT, exp'd with no
    max subtraction (scores are ~N(0,1); fp32 exp cannot overflow), causal
    diagonal masked by a triangular multiply.
  - attention output accumulates in the efficient o[q, d] orientation
    (lhsT = es[k, q-subblock 128], rhs = v[k, 65]): stationary = 128 queries,
    moving = 65, i.e. half the PE rows of the oT[d, q] orientation. The ones
    column yields the denominator as o[:, 64].
  - normalization is a per-partition tensor_scalar multiply by the
    reciprocal denominator (no partition broadcast needed), packing head
    pairs side by side; a PE transpose of [128, 128] blocks then restores the
    oT[c, q] layout the output projection needs as lhsT.
"""

import numpy as np
import ml_dtypes

import concourse.bass as bass
import concourse.mybir as mybir
import concourse.tile as tile
from concourse import bacc
from concourse.bass_utils import run_bass_kernel_spmd
from concourse.masks import make_upper_triangular

F32 = mybir.dt.float32
BF16 = mybir.dt.bfloat16
EMB = 1024
HEADS = 16
HD = 64
B = 4
S = 2048
NCORES = 8
HPC = 8           # heads per core
CD = HPC * HD     # 512 cols per core for each of q/k/v
NKB = S // 128    # 16 key blocks
NQC = S // 512    # 4 query chunks

_EXP = mybir.ActivationFunctionType.Exp


def _build_module():
    nc = bacc.Bacc("TRN2", target_bir_lowering=False, debug=False)
    xT = nc.declare_dram_parameter("xT", [EMB, S], BF16, isOutput=False)
    wq = nc.declare_dram_parameter("wq", [EMB, CD], BF16, isOutput=False)
    wk = nc.declare_dram_parameter("wk", [EMB, CD], BF16, isOutput=False)
    wv = nc.declare_dram_parameter("wv", [EMB, CD], BF16, isOutput=False)
    wp = nc.declare_dram_parameter("wp", [CD, EMB], BF16, isOutput=False)
    bias = nc.declare_dram_parameter("bias", [1, EMB], F32, isOutput=False)
    ones = nc.declare_dram_parameter("ones", [NKB, HPC], BF16, isOutput=False)
    ident = nc.declare_dram_parameter("ident", [128, 128], BF16, isOutput=False)
    y = nc.declare_dram_parameter("y", [S, EMB], BF16, isOutput=True)

    with tile.TileContext(nc) as tc:
        _body(tc, nc, xT, wq, wk, wv, wp, bias, ones, ident, y)
    nc.compile()
    return nc


def _body(tc, nc, xT, wq, wk, wv, wp, bias, ones, ident, y):
    from contextlib import ExitStack

    with ExitStack() as ctx:
        persist = ctx.enter_context(tc.tile_pool(name="persist", bufs=1))
        qt = persist.tile([128, 4, S], BF16, tag="qt")
        kt = persist.tile([128, 4, S], BF16, tag="kt")
        vx = persist.tile([128, NKB, HPC, HD + 1], BF16, tag="vx")

        # ones column for denominators (memset works for bf16)
        nc.gpsimd.memset(vx[:, :, :, HD : HD + 1], 1.0)
        # causal mask for diagonal blocks: tri[p, f] = 1.0 iff f >= p
        tri = persist.tile([128, 128], BF16, tag="tri")
        make_upper_triangular(nc, tri[:], val=1.0, diag=True)
        ident_sb = persist.tile([128, 128], BF16, tag="ident")
        nc.gpsimd.dma_start(out=ident_sb[:], in_=ident[:])

        # ---------------- Phase 1: QKV projections ----------------
        with ExitStack() as p1:
            xt_pool = p1.enter_context(tc.tile_pool(name="xt", bufs=2))
            w_pool = p1.enter_context(tc.tile_pool(name="w", bufs=4))
            wv_pool = p1.enter_context(tc.tile_pool(name="wvp", bufs=1))
            qkv_ps = p1.enter_context(
                tc.tile_pool(name="qkvps", bufs=4, space="PSUM")
            )

            wv_sb = wv_pool.tile([128, 8, CD], BF16, tag="wv")

            for half in range(2):
                t0 = half * 1024
                xt_sb = xt_pool.tile([128, 8, 1024], BF16, tag="xt")
                # load the n=0 token halves of every chunk first: the first
                # matmul group needs only those, halving the startup DMA gate
                for n2 in range(2):
                    for kc in range(8):
                        c0 = t0 + n2 * 512
                        nc.sync.dma_start(
                            out=xt_sb[:, kc, n2 * 512 : (n2 + 1) * 512],
                            in_=xT[kc * 128 : (kc + 1) * 128, c0 : c0 + 512],
                        )
                # qT and kT (transposed outputs)
                for qk, wdram, dst in ((0, wq, qt), (1, wk, kt)):
                    for m in range(4):
                        wt = w_pool.tile([128, 8, 128], BF16, tag="w")
                        nc.scalar.dma_start(
                            out=wt[:],
                            in_=wdram[:, m * 128 : (m + 1) * 128].rearrange(
                                "(c p) m -> p c m", p=128
                            ),
                        )
                        for n in range(2):
                            ps = qkv_ps.tile([128, 512], F32, tag="qkvps")
                            for kc in range(8):
                                nc.tensor.matmul(
                                    ps[:],
                                    lhsT=(wt[:, kc, :]),
                                    rhs=(xt_sb[:, kc, n * 512 : (n + 1) * 512]),
                                    start=(kc == 0),
                                    stop=(kc == 7),
                                )
                            col = t0 + n * 512
                            nc.vector.tensor_copy(
                                out=dst[:, m, col : col + 512], in_=ps[:]
                            )
                # v (natural orientation, strided into vx)
                if half == 0:
                    for kc in range(8):
                        nc.gpsimd.dma_start(
                            out=wv_sb[:, kc, :],
                            in_=wv[kc * 128 : (kc + 1) * 128, :],
                        )
                for tc8 in range(8):
                    tg = half * 8 + tc8
                    ps = qkv_ps.tile([128, 512], F32, tag="qkvps")
                    for kc in range(8):
                        nc.tensor.matmul(
                            ps[:],
                            lhsT=(xt_sb[:, kc, tc8 * 128 : (tc8 + 1) * 128]),
                            rhs=(wv_sb[:, kc, :]),
                            start=(kc == 0),
                            stop=(kc == 7),
                        )
                    nc.vector.tensor_copy(
                        out=vx[:, tg, :, 0:HD],
                        in_=ps[:].rearrange("p (h d) -> p h d", h=HPC),
                    )

        # ---------------- Phase 2+3: attention + output projection ----------------
        with ExitStack() as p2:
            misc = p2.enter_context(tc.tile_pool(name="misc", bufs=1))
            s_pool = p2.enter_context(tc.tile_pool(name="sps", bufs=2, space="PSUM"))
            o_pool = p2.enter_context(tc.tile_pool(name="ops", bufs=2, space="PSUM"))
            oT_ps_pool = p2.enter_context(
                tc.tile_pool(name="oTps", bufs=2, space="PSUM")
            )
            y_pool = p2.enter_context(tc.tile_pool(name="yps", bufs=2, space="PSUM"))
            e_pool = p2.enter_context(tc.tile_pool(name="es", bufs=5))
            r_pool = p2.enter_context(tc.tile_pool(name="recip", bufs=3))
            pair_pool = p2.enter_context(tc.tile_pool(name="pair", bufs=2))
            oT_pool = p2.enter_context(tc.tile_pool(name="oT", bufs=2))
            ysb_pool = p2.enter_context(tc.tile_pool(name="ysb", bufs=3))

            wp_sb = misc.tile([128, 4, EMB], BF16, tag="wp")
            nc.gpsimd.dma_start(
                out=wp_sb[:], in_=wp[:].rearrange("(c p) e -> p c e", p=128)
            )
            bias_sb = misc.tile([128, 1, EMB], F32, tag="bias")
            nc.gpsimd.dma_start(out=bias_sb[:], in_=bias[:].partition_broadcast(128))

            def make_y_group(oT_prev, qc_prev, tc4, ncol):
                def emit():
                    row = qc_prev * 512 + tc4 * 128
                    y_ps = y_pool.tile([128, 512], F32, tag="y")
                    for kc in range(4):
                        nc.tensor.matmul(
                            y_ps[:],
                            lhsT=(oT_prev[:, kc, tc4 * 128 : (tc4 + 1) * 128]),
                            rhs=(wp_sb[:, kc, ncol * 512 : (ncol + 1) * 512]),
                            start=(kc == 0),
                            stop=(kc == 3),
                        )
                    y_sb = ysb_pool.tile([128, 512], BF16, tag="ysb")
                    nc.vector.tensor_add(
                        y_sb[:],
                        y_ps[:],
                        bias_sb[:, 0, ncol * 512 : (ncol + 1) * 512],
                    )
                    nc.sync.dma_start(
                        out=y[row : row + 128, ncol * 512 : (ncol + 1) * 512],
                        in_=y_sb[:],
                    )

                return emit

            deferred_y = []
            for qc in range(NQC):
                oT = oT_pool.tile([128, 4, 512], BF16, tag="oT")
                kb_max = 4 * qc + 4
                o_sb = None
                for h in range(HPC):
                    m, hh = h // 2, h % 2
                    # full 2KB PSUM bank; sb regions at 512B strides. PSUM
                    # start=True zeroes the WHOLE bank (2KB zero region), so
                    # only the very first write into the bank may set it.
                    o_ps = o_pool.tile([128, 4, 128], F32, tag="ops")
                    # previous chunk's output projection fills the PE while
                    # this head's exp chain runs on the scalar engine
                    if deferred_y:
                        deferred_y.pop(0)()

                    def emit_av(kb, q0, es):
                        sb0 = q0 // 128
                        for sb in range(sb0, 4):
                            nc.tensor.matmul(
                                out=o_ps[:, sb, 0 : HD + 1],
                                lhsT=(es[:, (sb - sb0) * 128 : (sb - sb0 + 1) * 128]),
                                rhs=(vx[:, kb, h, :]),
                                start=(kb == 0 and sb == 0),
                                stop=(kb == 4 * qc + sb),
                                skip_group_check=True,
                            )

                    pending = None
                    for kb in range(kb_max):
                        r = kb * 128 - qc * 512
                        q0 = max(r, 0)
                        nq = 512 - q0
                        s_ps = s_pool.tile([128, 512], F32, tag="s")
                        es = e_pool.tile([128, 512], BF16, tag="es")
                        nc.tensor.matmul(
                            out=s_ps[:, 0:nq],
                            lhsT=(kt[hh * 64 : hh * 64 + 64, m, kb * 128 : (kb + 1) * 128]),
                            rhs=(qt[hh * 64 : hh * 64 + 64, m, qc * 512 + q0 : (qc + 1) * 512]),
                            start=True,
                            stop=True,
                        )
                        nc.scalar.activation(
                            out=es[:, 0:nq], in_=s_ps[:, 0:nq], func=_EXP
                        )
                        if r >= 0:
                            # diagonal block: mask the first 128 query columns
                            nc.vector.tensor_mul(es[:, 0:128], es[:, 0:128], tri[:])
                        # av matmuls run one block behind so the PE never
                        # waits on the exp of the block it just produced
                        if pending is not None:
                            emit_av(*pending)
                        pending = (kb, q0, es)
                    if pending is not None:
                        emit_av(*pending)
                    # normalize: per-partition multiply by 1/denominator
                    recip = r_pool.tile([128, 4], F32, tag="recip")
                    nc.vector.reciprocal(recip[:], o_ps[:, :, HD])
                    if hh == 0:
                        o_sb = pair_pool.tile([128, 4, 128], BF16, tag="pair")
                    for sb in range(4):
                        nc.vector.tensor_scalar_mul(
                            o_sb[:, sb, hh * 64 : hh * 64 + 64],
                            o_ps[:, sb, 0:HD],
                            recip[:, sb : sb + 1],
                        )
                    if hh == 1:
                        # head pair complete: transpose [128 q, 128 c] blocks
                        # back to the oT[c, q] layout the projection needs.
                        # Padded to a full 2KB bank; start only on first write.
                        oT_ps = oT_ps_pool.tile([128, 4, 256], BF16, tag="oTps")
                        for sb in range(4):
                            nc.tensor.matmul(
                                oT_ps[:, sb, 0:128],
                                lhsT=o_sb[:, sb, :],
                                rhs=ident_sb[:],
                                is_transpose=True,
                                start=(sb == 0),
                                stop=(sb == 3),
                                skip_group_check=True,
                            )
                        nc.vector.tensor_copy(
                            out=oT[:, m, :].rearrange("p (s q) -> p s q", s=4),
                            in_=oT_ps[:, :, 0:128],
                        )

                # defer this chunk's output projection into the next chunk's
                # head loop (emitted one group per head)
                assert not deferred_y
                deferred_y = [
                    make_y_group(oT, qc, tc4, ncol)
                    for tc4 in range(4)
                    for ncol in range(2)
                ]
            for emit in deferred_y:
                emit()


_MODULE = None


def _get_module():
    global _MODULE
    if _MODULE is None:
        _MODULE = _build_module()
    return _MODULE


def _bf16(a):
    return np.ascontiguousarray(np.asarray(a, dtype=np.float32)).astype(
        ml_dtypes.bfloat16
    )


def _make_in_maps(x, W_qkv, W_proj, b_proj):
    scale = np.float32(1.0 / np.sqrt(HD))
    bias_half = (np.asarray(b_proj, dtype=np.float32) * 0.5).reshape(1, EMB)
    in_maps = []
    for c in range(NCORES):
        b, hg = c // 2, c % 2
        cols = slice(hg * CD, (hg + 1) * CD)
        in_maps.append(
            {
                "xT": _bf16(np.asarray(x[b], dtype=np.float32).T),
                "wq": _bf16(W_qkv[:, 0:EMB][:, cols] * scale),
                "wk": _bf16(W_qkv[:, EMB : 2 * EMB][:, cols]),
                "wv": _bf16(W_qkv[:, 2 * EMB : 3 * EMB][:, cols]),
                "wp": _bf16(W_proj[cols, :]),
                "bias": bias_half.astype(np.float32),
                "ones": np.ones((NKB, HPC), dtype=ml_dtypes.bfloat16),
                "ident": np.eye(128, dtype=ml_dtypes.bfloat16),
            }
        )
    return in_maps


def kernel(x, W_qkv, W_proj, b_proj, _trace=False, _trace_kwargs=None):
    x = np.asarray(x, dtype=np.float32)
    W_qkv = np.asarray(W_qkv, dtype=np.float32)
    W_proj = np.asarray(W_proj, dtype=np.float32)
    b_proj = np.asarray(b_proj, dtype=np.float32)

    nc = _get_module()
    in_maps = _make_in_maps(x, W_qkv, W_proj, b_proj)
    res = run_bass_kernel_spmd(
        nc, in_maps, list(range(NCORES)), trace=_trace, **(_trace_kwargs or {})
    )
    out = np.empty((B, S, EMB), dtype=np.float32)
    for b in range(B):
        out[b] = np.asarray(res.results[2 * b]["y"], dtype=np.float32) + np.asarray(
            res.results[2 * b + 1]["y"], dtype=np.float32
        )
    if _trace:
        return out, res
    return out


# revision 12
# speedup vs baseline: 1.0818x; 1.0581x over previous
"""Causal attention block (B=4, S=2048, D=1024, H=16) on 8 Trainium2 NeuronCores.

Sharding: core c = (batch b = c//2, head-group hg = c%2 of 8 heads).
Each core computes QKV projection for its batch restricted to its heads'
columns, causal flash-style attention for its 8 heads, and a partial output
projection (its heads' rows of W_proj). Host sums the two partial outputs
per batch pair and returns the full [4, 2048, 1024] result.

All matmul operands are bf16 (PSUM accumulation stays fp32): on TRN2 the PE
processes one moving row per cycle regardless of dtype, but bf16 halves DMA
bytes, lifts the fp32r moving<256 penalty, and doubles 2-byte DVE ops.

Engine balance: the QKV + output projections are PE-bound; attention is
Activation-bound (the exp chain). The second token-half's QKV projection and
the per-chunk output projections are therefore deferred into a filler queue
drained one group per score-pair iteration during attention, so the PE chews
projection work while the scalar engine runs exp.

Layout choices (per core):
  - x arrives pre-transposed as xT [1024, 2048] so the embedding dim (the
    matmul contraction dim) is the SBUF partition dim.
  - q, k are produced transposed: qT/kT [512 cols, 2048 tokens] stored as
    [128, 4, 2048] tiles; head h lives in tile chunk h//2, partitions
    (h%2)*64..+64. 1/sqrt(hd) folded into W_q on the host.
  - v is produced in natural [token, col] orientation as [128, 16, 8, 65]
    (key-block, head, 64 v-cols + a ones column for softmax denominators).
  - scores are computed transposed, sT[k, q] = kT_block.T @ qT, into paired
    [128, 2, 512] PSUM tiles so one exp covers two key blocks (halving the
    activation-engine per-instruction overhead on full blocks), exp'd with no
    max subtraction (scores are ~N(0,1); fp32 exp cannot overflow), causal
    diagonal masked by a triangular multiply.
  - attention output accumulates in the efficient o[q, d] orientation
    (lhsT = es[k, q-subblock 128], rhs = v[k, 65]): stationary = 128 queries,
    moving = 65, i.e. half the PE rows of the oT[d, q] orientation. The ones
    column yields the denominator as o[:, 64]. PSUM start=True zeroes the
    whole 2KB bank, so only the first write into each bank sets it.
  - normalization is a per-partition tensor_scalar multiply by the
    reciprocal denominator (no partition broadcast needed), packing head
    pairs side by side; a PE transpose of [128, 128] blocks then restores the
    oT[c, q] layout the output projection needs as lhsT.
"""

from collections import deque

import numpy as np
import ml_dtypes

import concourse.bass as bass
import concourse.mybir as mybir
import concourse.tile as tile
from concourse import bacc
from concourse.bass_utils import run_bass_kernel_spmd
from concourse.masks import make_upper_triangular

F32 = mybir.dt.float32
BF16 = mybir.dt.bfloat16
EMB = 1024
HEADS = 16
HD = 64
B = 4
S = 2048
NCORES = 8
HPC = 8           # heads per core
CD = HPC * HD     # 512 cols per core for each of q/k/v
NKB = S // 128    # 16 key blocks
NQC = S // 512    # 4 query chunks

_EXP = mybir.ActivationFunctionType.Exp


def _build_module():
    nc = bacc.Bacc("TRN2", target_bir_lowering=False, debug=False)
    xT = nc.declare_dram_parameter("xT", [EMB, S], BF16, isOutput=False)
    wq = nc.declare_dram_parameter("wq", [EMB, CD], BF16, isOutput=False)
    wk = nc.declare_dram_parameter("wk", [EMB, CD], BF16, isOutput=False)
    wv = nc.declare_dram_parameter("wv", [EMB, CD], BF16, isOutput=False)
    wp = nc.declare_dram_parameter("wp", [CD, EMB], BF16, isOutput=False)
    bias = nc.declare_dram_parameter("bias", [1, EMB], F32, isOutput=False)
    ident = nc.declare_dram_parameter("ident", [128, 128], BF16, isOutput=False)
    y = nc.declare_dram_parameter("y", [S, EMB], BF16, isOutput=True)

    with tile.TileContext(nc) as tc:
        _body(tc, nc, xT, wq, wk, wv, wp, bias, ident, y)
    nc.compile()
    return nc


def _body(tc, nc, xT, wq, wk, wv, wp, bias, ident, y):
    from contextlib import ExitStack

    with ExitStack() as ctx:
        persist = ctx.enter_context(tc.tile_pool(name="persist", bufs=1))
        qt = persist.tile([128, 4, S], BF16, tag="qt")
        kt = persist.tile([128, 4, S], BF16, tag="kt")
        vx = persist.tile([128, NKB, HPC, HD + 1], BF16, tag="vx")
        tri = persist.tile([128, 128], BF16, tag="tri")
        ident_sb = persist.tile([128, 128], BF16, tag="ident")
        wp_sb = persist.tile([128, 4, EMB], BF16, tag="wp")
        bias_sb = persist.tile([128, 1, EMB], F32, tag="bias")

        # ones column for denominators; causal tri[p, f] = 1.0 iff f >= p
        nc.gpsimd.memset(vx[:, :, :, HD : HD + 1], 1.0)
        make_upper_triangular(nc, tri[:], val=1.0, diag=True)
        # pool-queue constant loads (pool engine is otherwise idle)
        nc.gpsimd.dma_start(out=ident_sb[:], in_=ident[:])
        nc.gpsimd.dma_start(
            out=wp_sb[:], in_=wp[:].rearrange("(c p) e -> p c e", p=128)
        )
        nc.gpsimd.dma_start(out=bias_sb[:], in_=bias[:].partition_broadcast(128))

        xt_pool = ctx.enter_context(tc.tile_pool(name="xt", bufs=2))
        w_pool = ctx.enter_context(tc.tile_pool(name="w", bufs=8))
        wv_pool = ctx.enter_context(tc.tile_pool(name="wvp", bufs=1))
        mm_ps = ctx.enter_context(tc.tile_pool(name="mmps", bufs=2, space="PSUM"))
        s_pool = ctx.enter_context(tc.tile_pool(name="sps", bufs=2, space="PSUM"))
        o_pool = ctx.enter_context(tc.tile_pool(name="ops", bufs=1, space="PSUM"))
        oT_ps_pool = ctx.enter_context(
            tc.tile_pool(name="oTps", bufs=1, space="PSUM")
        )
        e_pool = ctx.enter_context(tc.tile_pool(name="es", bufs=3))
        r_pool = ctx.enter_context(tc.tile_pool(name="recip", bufs=2))
        pair_pool = ctx.enter_context(tc.tile_pool(name="pair", bufs=2))
        oT_pool = ctx.enter_context(tc.tile_pool(name="oT", bufs=2))
        ysb_pool = ctx.enter_context(tc.tile_pool(name="ysb", bufs=2))

        # QKV weight tiles: loaded once, reused for both token halves.
        # Act-engine queue (idle during phase 1), ordered q-first to match
        # the first matmul groups.
        wv_sb = wv_pool.tile([128, 8, CD], BF16, tag="wv")
        w_tiles = {}
        for qk, wdram in ((0, wq), (1, wk)):
            for m in range(4):
                wt = w_pool.tile([128, 8, 128], BF16, tag="w")
                nc.scalar.dma_start(
                    out=wt[:],
                    in_=wdram[:, m * 128 : (m + 1) * 128].rearrange(
                        "(c p) m -> p c m", p=128
                    ),
                )
                w_tiles[(qk, m)] = wt

        xt_tiles = {}

        def load_xt(half, n2, engine):
            if half not in xt_tiles:
                xt_tiles[half] = xt_pool.tile(
                    [128, 8, 1024], BF16, tag="xt", name=f"xt{half}"
                )
            xt_sb = xt_tiles[half]
            for kc in range(8):
                c0 = half * 1024 + n2 * 512
                engine.dma_start(
                    out=xt_sb[:, kc, n2 * 512 : (n2 + 1) * 512],
                    in_=xT[kc * 128 : (kc + 1) * 128, c0 : c0 + 512],
                )

        def qk_group(half, qk, m, n):
            xt_sb = xt_tiles[half]
            wt = w_tiles[(qk, m)]
            dst = qt if qk == 0 else kt
            ps = mm_ps.tile([128, 512], F32, tag="mmps")
            for kc in range(8):
                nc.tensor.matmul(
                    ps[:],
                    lhsT=(wt[:, kc, :]),
                    rhs=(xt_sb[:, kc, n * 512 : (n + 1) * 512]),
                    start=(kc == 0),
                    stop=(kc == 7),
                )
            col = half * 1024 + n * 512
            nc.vector.tensor_copy(out=dst[:, m, col : col + 512], in_=ps[:])

        def v_group(half, tc8):
            xt_sb = xt_tiles[half]
            tg = half * 8 + tc8
            ps = mm_ps.tile([128, 512], F32, tag="mmps")
            for kc in range(8):
                nc.tensor.matmul(
                    ps[:],
                    lhsT=(xt_sb[:, kc, tc8 * 128 : (tc8 + 1) * 128]),
                    rhs=(wv_sb[:, kc, :]),
                    start=(kc == 0),
                    stop=(kc == 7),
                )
            nc.vector.tensor_copy(
                out=vx[:, tg, :, 0:HD],
                in_=ps[:].rearrange("p (h d) -> p h d", h=HPC),
            )

        # ---------------- Phase 1: first-half QKV (PE-bound) ----------------
        load_xt(0, 0, nc.sync)
        load_xt(0, 1, nc.gpsimd)
        for kc in range(8):
            nc.gpsimd.dma_start(
                out=wv_sb[:, kc, :], in_=wv[kc * 128 : (kc + 1) * 128, :]
            )
        for qk in (0, 1):
            for m in range(4):
                qk_group(0, qk, m, 0)
        for tc8 in range(4):
            v_group(0, tc8)
        for qk in (0, 1):
            for m in range(4):
                qk_group(0, qk, m, 1)
        for tc8 in range(4, 8):
            v_group(0, tc8)

        # second-half x loads + deferred QKV groups (drained during attention)
        load_xt(1, 0, nc.sync)
        load_xt(1, 1, nc.sync)
        filler = deque()
        for qk in (0, 1):
            for m in range(4):
                filler.append(lambda qk=qk, m=m: qk_group(1, qk, m, 0))
        for tc8 in range(4):
            filler.append(lambda tc8=tc8: v_group(1, tc8))
        for qk in (0, 1):
            for m in range(4):
                filler.append(lambda qk=qk, m=m: qk_group(1, qk, m, 1))
        for tc8 in range(4, 8):
            filler.append(lambda tc8=tc8: v_group(1, tc8))

        def make_y_group(oT_prev, qc_prev, tc4, ncol):
            def emit():
                row = qc_prev * 512 + tc4 * 128
                y_ps = mm_ps.tile([128, 512], F32, tag="mmps")
                for kc in range(4):
                    nc.tensor.matmul(
                        y_ps[:],
                        lhsT=(oT_prev[:, kc, tc4 * 128 : (tc4 + 1) * 128]),
                        rhs=(wp_sb[:, kc, ncol * 512 : (ncol + 1) * 512]),
                        start=(kc == 0),
                        stop=(kc == 3),
                    )
                y_sb = ysb_pool.tile([128, 512], BF16, tag="ysb")
                nc.vector.tensor_add(
                    y_sb[:],
                    y_ps[:],
                    bias_sb[:, 0, ncol * 512 : (ncol + 1) * 512],
                )
                nc.sync.dma_start(
                    out=y[row : row + 128, ncol * 512 : (ncol + 1) * 512],
                    in_=y_sb[:],
                )

            return emit

        # ---------------- Phase 2: attention (Act-bound) ----------------
        pop_gate = [0]

        def pop_filler():
            if filler and pop_gate[0] % 2 == 0:
                filler.popleft()()
            pop_gate[0] += 1

        for qc in range(NQC):
            oT = oT_pool.tile([128, 4, 512], BF16, tag="oT")
            kb_max = 4 * qc + 4
            o_sb = None
            for h in range(HPC):
                m, hh = h // 2, h % 2
                o_ps = o_pool.tile([128, 4, 128], F32, tag="ops")

                def emit_av(kb, q0, es, jj):
                    sb0 = q0 // 128
                    for sb in range(sb0, 4):
                        nc.tensor.matmul(
                            out=o_ps[:, sb, 0 : HD + 1],
                            lhsT=(
                                es[:, jj, (sb - sb0) * 128 : (sb - sb0 + 1) * 128]
                            ),
                            rhs=(vx[:, kb, h, :]),
                            start=(kb == 0 and sb == 0),
                            stop=(kb == 4 * qc + sb),
                            skip_group_check=True,
                        )

                pending = []
                for pj in range(kb_max // 2):
                    pop_filler()
                    s_ps = s_pool.tile([128, 2, 512], F32, tag="s")
                    es = e_pool.tile([128, 2, 512], BF16, tag="es")
                    nqs = []
                    for jj in range(2):
                        kb = 2 * pj + jj
                        r = kb * 128 - qc * 512
                        q0 = max(r, 0)
                        nq = 512 - q0
                        nqs.append((kb, q0, nq))
                        nc.tensor.matmul(
                            out=s_ps[:, jj, 0:nq],
                            lhsT=(
                                kt[hh * 64 : hh * 64 + 64, m, kb * 128 : (kb + 1) * 128]
                            ),
                            rhs=(
                                qt[
                                    hh * 64 : hh * 64 + 64,
                                    m,
                                    qc * 512 + q0 : (qc + 1) * 512,
                                ]
                            ),
                            start=True,
                            stop=True,
                        )
                    if nqs[0][2] == 512 and nqs[1][2] == 512:
                        # full pair: one exp over both banks
                        nc.scalar.activation(out=es[:], in_=s_ps[:], func=_EXP)
                    else:
                        for jj, (kb, q0, nq) in enumerate(nqs):
                            nc.scalar.activation(
                                out=es[:, jj, 0:nq], in_=s_ps[:, jj, 0:nq], func=_EXP
                            )
                    for jj, (kb, q0, nq) in enumerate(nqs):
                        if kb * 128 - qc * 512 >= 0:
                            # diagonal block: mask the first 128 query columns
                            nc.vector.tensor_mul(
                                es[:, jj, 0:128], es[:, jj, 0:128], tri[:]
                            )
                    # av matmuls run one pair behind so the PE never waits on
                    # the exp of the pair it just produced
                    for kb, q0, nq in pending:
                        emit_av(kb, q0, es_prev, kb & 1)
                    pending, es_prev = nqs, es
                for kb, q0, nq in pending:
                    emit_av(kb, q0, es_prev, kb & 1)
                # normalize: per-partition multiply by 1/denominator
                recip = r_pool.tile([128, 4], F32, tag="recip")
                nc.vector.reciprocal(recip[:], o_ps[:, :, HD])
                if hh == 0:
                    o_sb = pair_pool.tile([128, 4, 128], BF16, tag="pair")
                for sb in range(4):
                    nc.vector.tensor_scalar_mul(
                        o_sb[:, sb, hh * 64 : hh * 64 + 64],
                        o_ps[:, sb, 0:HD],
                        recip[:, sb : sb + 1],
                    )
                if hh == 1:
                    # fill the PE while the normalize chain runs on the DVE
                    pop_filler()
                    # head pair complete: transpose [128 q, 128 c] blocks
                    # back to the oT[c, q] layout the projection needs.
                    oT_ps = oT_ps_pool.tile([128, 4, 256], BF16, tag="oTps")
                    for sb in range(4):
                        nc.tensor.matmul(
                            oT_ps[:, sb, 0:128],
                            lhsT=o_sb[:, sb, :],
                            rhs=ident_sb[:],
                            is_transpose=True,
                            start=(sb == 0),
                            stop=(sb == 3),
                            skip_group_check=True,
                        )
                    nc.vector.tensor_copy(
                        out=oT[:, m, :].rearrange("p (s q) -> p s q", s=4),
                        in_=oT_ps[:, :, 0:128],
                    )

            # defer this chunk's output projection into the filler queue
            for tc4 in range(4):
                for ncol in range(2):
                    filler.append(make_y_group(oT, qc, tc4, ncol))
        while filler:
            filler.popleft()()


_MODULE = None


def _get_module():
    global _MODULE
    if _MODULE is None:
        _MODULE = _build_module()
    return _MODULE


def _bf16(a):
    return np.ascontiguousarray(np.asarray(a, dtype=np.float32)).astype(
        ml_dtypes.bfloat16
    )


def _make_in_maps(x, W_qkv, W_proj, b_proj):
    scale = np.float32(1.0 / np.sqrt(HD))
    bias_half = (np.asarray(b_proj, dtype=np.float32) * 0.5).reshape(1, EMB)
    in_maps = []
    for c in range(NCORES):
        b, hg = c // 2, c % 2
        cols = slice(hg * CD, (hg + 1) * CD)
        in_maps.append(
            {
                "xT": _bf16(np.asarray(x[b], dtype=np.float32).T),
                "wq": _bf16(W_qkv[:, 0:EMB][:, cols] * scale),
                "wk": _bf16(W_qkv[:, EMB : 2 * EMB][:, cols]),
                "wv": _bf16(W_qkv[:, 2 * EMB : 3 * EMB][:, cols]),
                "wp": _bf16(W_proj[cols, :]),
                "bias": bias_half.astype(np.float32),
                "ident": np.eye(128, dtype=ml_dtypes.bfloat16),
            }
        )
    return in_maps


def kernel(x, W_qkv, W_proj, b_proj, _trace=False, _trace_kwargs=None):
    x = np.asarray(x, dtype=np.float32)
    W_qkv = np.asarray(W_qkv, dtype=np.float32)
    W_proj = np.asarray(W_proj, dtype=np.float32)
    b_proj = np.asarray(b_proj, dtype=np.float32)

    nc = _get_module()
    in_maps = _make_in_maps(x, W_qkv, W_proj, b_proj)
    res = run_bass_kernel_spmd(
        nc, in_maps, list(range(NCORES)), trace=_trace, **(_trace_kwargs or {})
    )
    out = np.empty((B, S, EMB), dtype=np.float32)
    for b in range(B):
        out[b] = np.asarray(res.results[2 * b]["y"], dtype=np.float32) + np.asarray(
            res.results[2 * b + 1]["y"], dtype=np.float32
        )
    if _trace:
        return out, res
    return out


# revision 16
# speedup vs baseline: 1.1465x; 1.0599x over previous
"""Causal attention block (B=4, S=2048, D=1024, H=16) on 8 Trainium2 NeuronCores.

Sharding: core c = (batch b = c//2, head-group hg = c%2 of 8 heads).
Each core computes QKV projection for its batch restricted to its heads'
columns, causal flash-style attention for its 8 heads, and a partial output
projection (its heads' rows of W_proj). Host sums the two partial outputs
per batch pair and returns the full [4, 2048, 1024] result.

All matmul operands are bf16 (PSUM accumulation stays fp32): on TRN2 the PE
processes one moving row per cycle regardless of dtype, but bf16 halves DMA
bytes, lifts the fp32r moving<256 penalty, and doubles 2-byte DVE ops.

Engine balance: projections (QKV + output) are PE-bound; attention is
Activation-bound (the exp chain). Attention starts as soon as the first
head's q/k columns and the first four key blocks of v exist; every other
projection group lives in an ordered filler queue drained one group per
score-pair iteration (with forced draining to satisfy data dependencies), so
the PE chews projection work whenever the scalar engine is the attention
rate limiter and ideally never idles.

Layout choices (per core):
  - x arrives pre-transposed as xT [1024, 2048] so the embedding dim (the
    matmul contraction dim) is the SBUF partition dim.
  - q, k are produced transposed: qT/kT [512 cols, 2048 tokens] stored as
    [128, 4, 2048] tiles; head h lives in tile chunk h//2, partitions
    (h%2)*64..+64. 1/sqrt(hd) folded into W_q on the host.
  - v is produced in natural [token, col] orientation as [128, 16, 8, 65]
    (key-block, head, 64 v-cols + a ones column for softmax denominators).
  - scores are computed transposed, sT[k, q] = kT_block.T @ qT, into paired
    [128, 2, 512] PSUM tiles so one exp covers two full key blocks (halving
    the activation-engine per-instruction overhead), exp'd with no max
    subtraction (scores are ~N(0,1); fp32 exp cannot overflow), causal
    diagonal masked by a triangular multiply.
  - attention output accumulates in the efficient o[q, d] orientation
    (lhsT = es[k, q-subblock 128], rhs = v[k, 65]): stationary = 128 queries,
    moving = 65, i.e. half the PE rows of the oT[d, q] orientation. The ones
    column yields the denominator as o[:, 64]. PSUM start=True zeroes the
    whole 2KB bank, so only the first write into each bank sets it.
  - normalization is a per-partition tensor_scalar multiply by the
    reciprocal denominator (no partition broadcast needed), packing head
    pairs side by side; a PE transpose of [128, 128] blocks then restores the
    oT[c, q] layout the output projection needs as lhsT.
"""

from collections import deque

import numpy as np
import ml_dtypes

import concourse.bass as bass
import concourse.mybir as mybir
import concourse.tile as tile
from concourse import bacc
from concourse.bass_utils import run_bass_kernel_spmd
from concourse.masks import make_upper_triangular

F32 = mybir.dt.float32
BF16 = mybir.dt.bfloat16
EMB = 1024
HEADS = 16
HD = 64
B = 4
S = 2048
NCORES = 8
HPC = 8           # heads per core
CD = HPC * HD     # 512 cols per core for each of q/k/v
NKB = S // 128    # 16 key blocks
NQC = S // 512    # 4 query chunks

_EXP = mybir.ActivationFunctionType.Exp


def _build_module():
    nc = bacc.Bacc("TRN2", target_bir_lowering=False, debug=False)
    xT = nc.declare_dram_parameter("xT", [EMB, S], BF16, isOutput=False)
    wq = nc.declare_dram_parameter("wq", [EMB, CD], BF16, isOutput=False)
    wk = nc.declare_dram_parameter("wk", [EMB, CD], BF16, isOutput=False)
    wv = nc.declare_dram_parameter("wv", [EMB, CD], BF16, isOutput=False)
    wp = nc.declare_dram_parameter("wp", [CD, EMB], BF16, isOutput=False)
    bias = nc.declare_dram_parameter("bias", [1, EMB], F32, isOutput=False)
    ident = nc.declare_dram_parameter("ident", [128, 128], BF16, isOutput=False)
    y = nc.declare_dram_parameter("y", [S, EMB], BF16, isOutput=True)

    with tile.TileContext(nc) as tc:
        _body(tc, nc, xT, wq, wk, wv, wp, bias, ident, y)
    nc.compile()
    return nc


def _body(tc, nc, xT, wq, wk, wv, wp, bias, ident, y):
    from contextlib import ExitStack

    with ExitStack() as ctx:
        persist = ctx.enter_context(tc.tile_pool(name="persist", bufs=1))
        qt = persist.tile([128, 4, S], BF16, tag="qt")
        kt = persist.tile([128, 4, S], BF16, tag="kt")
        vx = persist.tile([128, NKB, HPC, HD + 1], BF16, tag="vx")
        tri = persist.tile([128, 128], BF16, tag="tri")
        ident_sb = persist.tile([128, 128], BF16, tag="ident")
        wp_sb = persist.tile([128, 4, EMB], BF16, tag="wp")
        bias_sb = persist.tile([128, 1, EMB], F32, tag="bias")

        # ones column for denominators; causal tri[p, f] = 1.0 iff f >= p
        nc.gpsimd.memset(vx[:, :, :, HD : HD + 1], 1.0)
        make_upper_triangular(nc, tri[:], val=1.0, diag=True)

        xt_pool = ctx.enter_context(tc.tile_pool(name="xt", bufs=2))
        w_pool = ctx.enter_context(tc.tile_pool(name="w", bufs=8))
        wv_pool = ctx.enter_context(tc.tile_pool(name="wvp", bufs=1))
        mm_ps = ctx.enter_context(tc.tile_pool(name="mmps", bufs=2, space="PSUM"))
        s_pool = ctx.enter_context(tc.tile_pool(name="sps", bufs=2, space="PSUM"))
        o_pool = ctx.enter_context(tc.tile_pool(name="ops", bufs=1, space="PSUM"))
        oT_ps_pool = ctx.enter_context(
            tc.tile_pool(name="oTps", bufs=1, space="PSUM")
        )
        e_pool = ctx.enter_context(tc.tile_pool(name="es", bufs=3))
        r_pool = ctx.enter_context(tc.tile_pool(name="recip", bufs=2))
        pair_pool = ctx.enter_context(tc.tile_pool(name="pair", bufs=2))
        oT_pool = ctx.enter_context(tc.tile_pool(name="oT", bufs=2))
        ysb_pool = ctx.enter_context(tc.tile_pool(name="ysb", bufs=2))

        # ---- input loads, spread across the four DMA-issuing engines ----
        # SP: all xT tiles (first-needed first). Act: wq tiles. DVE: wk tiles.
        # Pool: wv, then constants needed only later (ident/wp/bias).
        xt_tiles = {
            0: xt_pool.tile([128, 8, 1024], BF16, tag="xt", name="xt0"),
            1: xt_pool.tile([128, 8, 1024], BF16, tag="xt", name="xt1"),
        }

        def load_xt(half, n2, engine):
            xt_sb = xt_tiles[half]
            for kc in range(8):
                c0 = half * 1024 + n2 * 512
                engine.dma_start(
                    out=xt_sb[:, kc, n2 * 512 : (n2 + 1) * 512],
                    in_=xT[kc * 128 : (kc + 1) * 128, c0 : c0 + 512],
                )

        load_xt(0, 0, nc.sync)
        w_tiles = {}

        def load_w(qk, m, eng):
            wdram = wq if qk == 0 else wk
            wt = w_pool.tile([128, 8, 128], BF16, tag="w", name=f"w{qk}{m}")
            eng.dma_start(
                out=wt[:],
                in_=wdram[:, m * 128 : (m + 1) * 128].rearrange(
                    "(c p) m -> p c m", p=128
                ),
            )
            w_tiles[(qk, m)] = wt

        # first two head-pairs' projection weights on the Act queue (fast,
        # needed first); the rest behind wv on the cheap Pool queue
        for m in range(2):
            load_w(0, m, nc.scalar)
            load_w(1, m, nc.scalar)
        wv_sb = wv_pool.tile([128, 8, CD], BF16, tag="wv")
        for kc in range(8):
            nc.gpsimd.dma_start(
                out=wv_sb[:, kc, :], in_=wv[kc * 128 : (kc + 1) * 128, :]
            )
        for m in range(2, 4):
            load_w(0, m, nc.gpsimd)
            load_w(1, m, nc.gpsimd)
        load_xt(0, 1, nc.gpsimd)
        load_xt(1, 0, nc.sync)
        load_xt(1, 1, nc.sync)
        nc.gpsimd.dma_start(out=ident_sb[:], in_=ident[:])
        nc.gpsimd.dma_start(
            out=wp_sb[:], in_=wp[:].rearrange("(c p) e -> p c e", p=128)
        )
        nc.gpsimd.dma_start(out=bias_sb[:], in_=bias[:].partition_broadcast(128))

        # ---- projection group emitters ----
        def qk_group(half, qk, m, n):
            xt_sb = xt_tiles[half]
            wt = w_tiles[(qk, m)]
            dst = qt if qk == 0 else kt
            ps = mm_ps.tile([128, 512], F32, tag="mmps", name="qkps")
            for kc in range(8):
                nc.tensor.matmul(
                    ps[:],
                    lhsT=(wt[:, kc, :]),
                    rhs=(xt_sb[:, kc, n * 512 : (n + 1) * 512]),
                    start=(kc == 0),
                    stop=(kc == 7),
                )
            col = half * 1024 + n * 512
            nc.vector.tensor_copy(out=dst[:, m, col : col + 512], in_=ps[:])

        def v_group(half, tc8):
            xt_sb = xt_tiles[half]
            tg = half * 8 + tc8
            ps = mm_ps.tile([128, 512], F32, tag="mmps", name="vps")
            for kc in range(8):
                nc.tensor.matmul(
                    ps[:],
                    lhsT=(xt_sb[:, kc, tc8 * 128 : (tc8 + 1) * 128]),
                    rhs=(wv_sb[:, kc, :]),
                    start=(kc == 0),
                    stop=(kc == 7),
                )
            nc.vector.tensor_copy(
                out=vx[:, tg, :, 0:HD],
                in_=ps[:].rearrange("p (h d) -> p h d", h=HPC),
            )

        def make_y_group(oT_prev, qc_prev, tc4, ncol):
            def emit():
                row = qc_prev * 512 + tc4 * 128
                y_ps = mm_ps.tile([128, 512], F32, tag="mmps", name="yps")
                for kc in range(4):
                    nc.tensor.matmul(
                        y_ps[:],
                        lhsT=(oT_prev[:, kc, tc4 * 128 : (tc4 + 1) * 128]),
                        rhs=(wp_sb[:, kc, ncol * 512 : (ncol + 1) * 512]),
                        start=(kc == 0),
                        stop=(kc == 3),
                    )
                y_sb = ysb_pool.tile([128, 512], BF16, tag="ysb", name="ysb")
                nc.vector.tensor_add(
                    y_sb[:],
                    y_ps[:],
                    bias_sb[:, 0, ncol * 512 : (ncol + 1) * 512],
                )
                nc.sync.dma_start(
                    out=y[row : row + 128, ncol * 512 : (ncol + 1) * 512],
                    in_=y_sb[:],
                )

            return emit

        # Ordered projection-group queue: q/k token-slice groups (m-ascending
        # per chunk so heads unblock progressively), then deferred output
        # projections. gate[(qc, m)] = count that must be emitted before
        # attention chunk qc head-pair m may run. v groups are a separate
        # queue drained at AV-emission granularity (vgate = key block index).
        G = []
        gate = {}
        for qc in range(NQC):
            half, n = qc // 2, qc % 2
            for m in range(4):
                G.append(("qk", half, 0, m, n))
                G.append(("qk", half, 1, m, n))
                gate[(qc, m)] = len(G)

        filler = deque(G)
        drained = [0]
        vq = deque((kb // 8, kb % 8) for kb in range(NKB))
        v_drained = [0]

        def pop_filler():
            if not filler:
                return False
            item = filler.popleft()
            if callable(item):
                item()
            else:
                qk_group(item[1], item[2], item[3], item[4])
            drained[0] += 1
            return True

        def drain_to(idx):
            while drained[0] < idx:
                pop_filler()

        def drain_v_to(kb_hi):
            while v_drained[0] <= kb_hi and vq:
                half, tc8 = vq.popleft()
                v_group(half, tc8)
                v_drained[0] += 1

        # ---------------- attention (Act-bound) + filler drain ----------------
        it = [0]
        for qc in range(NQC):
            oT = oT_pool.tile([128, 4, 512], BF16, tag="oT")
            kb_max = 4 * qc + 4
            o_sb = None
            for h in range(HPC):
                m, hh = h // 2, h % 2
                drain_to(gate[(qc, m)])
                o_ps = o_pool.tile([128, 4, 128], F32, tag="ops")

                def emit_av(kb, q0, es, jj):
                    sb0 = q0 // 128
                    for sb in range(sb0, 4):
                        nc.tensor.matmul(
                            out=o_ps[:, sb, 0 : HD + 1],
                            lhsT=(
                                es[:, jj, (sb - sb0) * 128 : (sb - sb0 + 1) * 128]
                            ),
                            rhs=(vx[:, kb, h, :]),
                            start=(kb == 0 and sb == 0),
                            stop=(kb == 4 * qc + sb),
                            skip_group_check=True,
                        )

                pending = []
                for pj in range(kb_max // 2):
                    # pace the filler: sparse during early chunks (the forced
                    # drains front-load them anyway), dense in the last chunk
                    it[0] += 1
                    if qc == NQC - 1 or it[0] % 2 == 0:
                        pop_filler()
                    s_ps = s_pool.tile([128, 2, 512], F32, tag="s")
                    es = e_pool.tile([128, 2, 512], BF16, tag="es")
                    nqs = []
                    for jj in range(2):
                        kb = 2 * pj + jj
                        r = kb * 128 - qc * 512
                        q0 = max(r, 0)
                        nq = 512 - q0
                        nqs.append((kb, q0, nq))
                        nc.tensor.matmul(
                            out=s_ps[:, jj, 0:nq],
                            lhsT=(
                                kt[hh * 64 : hh * 64 + 64, m, kb * 128 : (kb + 1) * 128]
                            ),
                            rhs=(
                                qt[
                                    hh * 64 : hh * 64 + 64,
                                    m,
                                    qc * 512 + q0 : (qc + 1) * 512,
                                ]
                            ),
                            start=True,
                            stop=True,
                        )
                    if nqs[0][2] == 512 and nqs[1][2] == 512:
                        # full pair: one exp over both banks
                        nc.scalar.activation(out=es[:], in_=s_ps[:], func=_EXP)
                    else:
                        for jj, (kb, q0, nq) in enumerate(nqs):
                            nc.scalar.activation(
                                out=es[:, jj, 0:nq], in_=s_ps[:, jj, 0:nq], func=_EXP
                            )
                    for jj, (kb, q0, nq) in enumerate(nqs):
                        if kb * 128 - qc * 512 >= 0:
                            # diagonal block: mask the first 128 query columns
                            nc.vector.tensor_mul(
                                es[:, jj, 0:128], es[:, jj, 0:128], tri[:]
                            )
                    # av matmuls run one pair behind so the PE never waits on
                    # the exp of the pair it just produced
                    if pending:
                        drain_v_to(pending[-1][0])
                    for kb, q0, nq in pending:
                        emit_av(kb, q0, es_prev, kb & 1)
                    pending, es_prev = nqs, es
                if pending:
                    drain_v_to(pending[-1][0])
                for kb, q0, nq in pending:
                    emit_av(kb, q0, es_prev, kb & 1)
                # normalize: per-partition multiply by 1/denominator
                recip = r_pool.tile([128, 4], F32, tag="recip")
                nc.vector.reciprocal(recip[:], o_ps[:, :, HD])
                if hh == 0:
                    o_sb = pair_pool.tile([128, 4, 128], BF16, tag="pair")
                for sb in range(4):
                    nc.vector.tensor_scalar_mul(
                        o_sb[:, sb, hh * 64 : hh * 64 + 64],
                        o_ps[:, sb, 0:HD],
                        recip[:, sb : sb + 1],
                    )
                if hh == 1:
                    # fill the PE while the normalize chain runs on the DVE
                    pop_filler()
                    # head pair complete: transpose [128 q, 128 c] blocks
                    # back to the oT[c, q] layout the projection needs.
                    oT_ps = oT_ps_pool.tile([128, 4, 256], BF16, tag="oTps")
                    for sb in range(4):
                        nc.tensor.matmul(
                            oT_ps[:, sb, 0:128],
                            lhsT=o_sb[:, sb, :],
                            rhs=ident_sb[:],
                            is_transpose=True,
                            start=(sb == 0),
                            stop=(sb == 3),
                            skip_group_check=True,
                        )
                    nc.vector.tensor_copy(
                        out=oT[:, m, :].rearrange("p (s q) -> p s q", s=4),
                        in_=oT_ps[:, :, 0:128],
                    )

            # defer this chunk's output projection into the filler queue
            for tc4 in range(4):
                for ncol in range(2):
                    filler.append(make_y_group(oT, qc, tc4, ncol))
        while filler:
            pop_filler()


_MODULE = None


def _get_module():
    global _MODULE
    if _MODULE is None:
        _MODULE = _build_module()
    return _MODULE


def _bf16(a):
    return np.ascontiguousarray(np.asarray(a, dtype=np.float32)).astype(
        ml_dtypes.bfloat16
    )


def _make_in_maps(x, W_qkv, W_proj, b_proj):
    scale = np.float32(1.0 / np.sqrt(HD))
    bias_half = (np.asarray(b_proj, dtype=np.float32) * 0.5).reshape(1, EMB)
    in_maps = []
    for c in range(NCORES):
        b, hg = c // 2, c % 2
        cols = slice(hg * CD, (hg + 1) * CD)
        in_maps.append(
            {
                "xT": _bf16(np.asarray(x[b], dtype=np.float32).T),
                "wq": _bf16(W_qkv[:, 0:EMB][:, cols] * scale),
                "wk": _bf16(W_qkv[:, EMB : 2 * EMB][:, cols]),
                "wv": _bf16(W_qkv[:, 2 * EMB : 3 * EMB][:, cols]),
                "wp": _bf16(W_proj[cols, :]),
                "bias": bias_half.astype(np.float32),
                "ident": np.eye(128, dtype=ml_dtypes.bfloat16),
            }
        )
    return in_maps


def kernel(x, W_qkv, W_proj, b_proj, _trace=False, _trace_kwargs=None):
    x = np.asarray(x, dtype=np.float32)
    W_qkv = np.asarray(W_qkv, dtype=np.float32)
    W_proj = np.asarray(W_proj, dtype=np.float32)
    b_proj = np.asarray(b_proj, dtype=np.float32)

    nc = _get_module()
    in_maps = _make_in_maps(x, W_qkv, W_proj, b_proj)
    res = run_bass_kernel_spmd(
        nc, in_maps, list(range(NCORES)), trace=_trace, **(_trace_kwargs or {})
    )
    out = np.empty((B, S, EMB), dtype=np.float32)
    for b in range(B):
        out[b] = np.asarray(res.results[2 * b]["y"], dtype=np.float32) + np.asarray(
            res.results[2 * b + 1]["y"], dtype=np.float32
        )
    if _trace:
        return out, res
    return out
